# revision 20
# baseline (speedup 1.0000x reference)
"""EpiPINN loss kernel for 8 Trainium2 NeuronCores (Bass/Tile).

Computes: 6-layer tanh MLP (1->512x5->5) over 8192 collocation points,
softmax -> SEIRD components y, Caputo L1 fractional derivative (lower
triangular Toeplitz [8191x8191] @ dpsi), SEIRD residual, scalar MSE loss.

Distribution: data-parallel MLP over rows (1024/core); AllGather of the
folded (y, dpsi) blocks; Toeplitz matmul sharded by output rows with the
strided assignment I = 8q + d (mod-8 interleave balances the triangle);
scalar AllReduce of the partial loss.

SPMD note: all 8 cores run one program. Per-core behavior enters through
input data only: the Caputo kernel vector is computed from a core-shifted
iota (w1_d[m] = w1[m + 128d]), which makes the 64-diagonal Toeplitz loop
core-uniform; one dynamic-offset AP extracts the core's assigned y blocks.
"""

import math

import numpy as np

H = 512
DEPTH = 6
N = 8192
DT = 0.1
MIN_ALPHA = 0.6
NCORES = 8
ROWS = N // NCORES          # 1024 rows per core
NB = N // 128               # 64 global 128-row blocks
NQ = NB // NCORES           # 8 out-blocks per core
WB = 8320                   # wbuf length = 128 * 65  (shifted-kernel values)
WBC = 65                    # wbuf free cols per partition
WMC = 128 * 64              # Wmega columns: diagonals m'' = 0..63
KT = H // 128               # 4 contraction tiles
RLOC = ROWS + 4             # 1028 rows per core (overlap rows for dpsi;
                            # padded so all chunks are 4-aligned for fp32r)
CHUNKS = ((0, 344), (344, 344), (688, 340))  # free chunks (<=512 psum)

_CACHE = {}


def _lgamma_coeffs(deg=7):
    # least-squares poly fit of lgamma on [1.0, 1.4] (alpha in [0.6, 1.0])
    x = np.linspace(1.0, 1.4, 2001)
    y = np.array([math.lgamma(v) for v in x])
    c = np.polyfit(x, y, deg)
    return c  # highest power first


def _build():
    from ml_dtypes import bfloat16 as ml_bf16
    import concourse.bass as bass
    import concourse.tile as tile
    from concourse import bacc, mybir

    f32 = mybir.dt.float32
    f32r = mybir.dt.float32r
    bf16 = mybir.dt.bfloat16
    f16 = mybir.dt.float16
    i32 = mybir.dt.int32
    AF = mybir.ActivationFunctionType
    OP = mybir.AluOpType

    nc = bacc.Bacc("TRN2", target_bir_lowering=False, debug=False,
                   num_devices=NCORES)

    # ---- kernel I/O ----
    tsh = nc.dram_tensor("tsh", [1, RLOC], f32, kind="ExternalInput")
    win = nc.dram_tensor("win", [1, H], f32, kind="ExternalInput")
    binp = nc.dram_tensor("binp", [128, KT], f32, kind="ExternalInput")
    whp = nc.dram_tensor("whp", [128, (DEPTH - 1) * KT * H], f32,
                         kind="ExternalInput")
    bhp = nc.dram_tensor("bhp", [128, (DEPTH - 1) * KT], f32,
                         kind="ExternalInput")
    woutp = nc.dram_tensor("woutp", [128, KT * 5], f32, kind="ExternalInput")
    bout5 = nc.dram_tensor("bout5", [5, 1], f32, kind="ExternalInput")
    params = nc.dram_tensor("params", [1, 8], f32, kind="ExternalInput")
    coref = nc.dram_tensor("coref", [1, 2], f32, kind="ExternalInput")
    out_d = nc.dram_tensor("out", [1, 1], f32, kind="ExternalOutput")

    ident5_d = nc.inline_tensor(np.eye(5, dtype=np.float32), name="ident5")
    j128_d = nc.inline_tensor(
        np.eye(128, dtype=np.float32)[::-1].copy().astype(ml_bf16),
        name="j128")

    lg = _lgamma_coeffs()

    with tile.TileContext(nc, num_cores=NCORES) as tc:
        with (
            tc.tile_pool(name="dram", bufs=1, space="DRAM") as dram,
            tc.tile_pool(name="const", bufs=1) as cpool,
            tc.tile_pool(name="acts", bufs=1) as apool,
            tc.tile_pool(name="small", bufs=1) as spool,
        ):
            # ------- DRAM scratch -------
            wbuf_dram = dram.tile([WB], bf16)
            cc2_in = dram.tile([128 * NCORES, 40], f32)
            rs_out = dram.tile([128, 40], f32)

            # ------- load inputs to SBUF -------
            tsh_sb = cpool.tile([1, RLOC], f32r)
            nc.gpsimd.dma_start(tsh_sb[:], tsh.ap())
            win_sb = cpool.tile([1, H], f32r)
            nc.gpsimd.dma_start(win_sb[:], win.ap())
            binp_sb = cpool.tile([128, KT], f32)
            nc.sync.dma_start(binp_sb[:], binp.ap())
            bhp_sb = cpool.tile([128, (DEPTH - 1) * KT], f32)
            nc.sync.dma_start(bhp_sb[:], bhp.ap())
            woutp_sb = cpool.tile([128, KT * 5], f16)
            nc.gpsimd.dma_start(woutp_sb[:], woutp.ap())
            bout5_sb = cpool.tile([5, 1], f32)
            nc.sync.dma_start(bout5_sb[:], bout5.ap())
            par_sb = cpool.tile([1, 8], f32)
            nc.sync.dma_start(par_sb[:], params.ap())
            coref_sb = cpool.tile([1, 2], f32)
            nc.sync.dma_start(coref_sb[:], coref.ap())
            ident5_sb = cpool.tile([5, 5], f32)
            nc.sync.dma_start(ident5_sb[:], ident5_d.ap())
            j128_sb = cpool.tile([128, 128], bf16)
            nc.sync.dma_start(j128_sb[:], j128_d.ap())

            wh_sb = []
            for l in range(DEPTH - 1):
                w = cpool.tile([128, KT * H], f16, tag=f"wh{l}")
                nc.gpsimd.dma_start(w[:], whp.ap()[:, l * KT * H:(l + 1) * KT * H])
                wh_sb.append(w)

            # ------- P1: scalar params (ln/exp table set) -------
            # softplus(raw) = ln(1 + exp(raw)) on params[0, 0:4]
            sp_e = spool.tile([1, 8], f32, tag="sp")
            act_late = []
            a = nc.scalar.activation(sp_e[0:1, 0:4], par_sb[0:1, 0:4], AF.Exp)
            act_late.append(a)
            nc.vector.tensor_scalar_add(sp_e[0:1, 0:4], sp_e[0:1, 0:4], 1.0)
            sp = spool.tile([1, 8], f32, tag="sp2")
            nc.scalar.activation(sp[0:1, 0:4], sp_e[0:1, 0:4], AF.Ln)
            # sp[0,0:4] = beta, sigma, gamma, mu

            # alpha = 0.6 + 0.4 * sigmoid(z_alpha);  sigmoid = 1/(1+exp(-z))
            alp = spool.tile([1, 4], f32, tag="alp")
            nc.scalar.activation(alp[0:1, 0:1], par_sb[0:1, 4:5], AF.Exp,
                                 scale=-1.0)
            nc.vector.tensor_scalar_add(alp[0:1, 0:1], alp[0:1, 0:1], 1.0)
            nc.vector.reciprocal(alp[0:1, 1:2], alp[0:1, 0:1])
            # alpha in alp[0,2]
            nc.vector.tensor_scalar(alp[0:1, 2:3], alp[0:1, 1:2],
                                    1.0 - MIN_ALPHA, MIN_ALPHA,
                                    OP.mult, OP.add)
            # e = 1 - alpha in alp[0,3]
            nc.vector.tensor_scalar(alp[0:1, 3:4], alp[0:1, 2:3],
                                    -1.0, 1.0, OP.mult, OP.add)

            # early broadcast: [e = 1-alpha, iota shift] (unblocks wbuf)
            e2 = spool.tile([1, 2], f32, tag="e2")
            nc.vector.tensor_copy(e2[0:1, 0:1], alp[0:1, 3:4])
            nc.vector.tensor_copy(e2[0:1, 1:2], coref_sb[0:1, 0:1])
            eb = cpool.tile([128, 2], f32)
            nc.gpsimd.partition_broadcast(eb[:], e2[0:1, :])
            e128 = eb[:, 0:1]
            shift128 = eb[:, 1:2]

            # lnGamma(2 - alpha) via Horner; x = 2 - alpha = 1 + e
            lgm = spool.tile([1, 2], f32, tag="lgm")
            xg = alp[0:1, 3:4]  # use e: x = 1 + e -> fold the +1 into coeffs?
            # evaluate directly in x = 1 + e by shifting: p(x), x = e + 1.
            # g = c0; g = g*x + ck  ... compute x first:
            nc.vector.tensor_scalar_add(lgm[0:1, 1:2], xg, 1.0)  # x
            nc.vector.memset(lgm[0:1, 0:1], float(lg[0]))
            for k in range(1, len(lg)):
                nc.vector.tensor_tensor(lgm[0:1, 0:1], lgm[0:1, 0:1],
                                        lgm[0:1, 1:2], OP.mult)
                nc.vector.tensor_scalar_add(lgm[0:1, 0:1], lgm[0:1, 0:1],
                                            float(lg[k]))

            # C = exp(-alpha*ln(DT) - lnGamma) ; ln(0.1) const
            cc_s = spool.tile([1, 2], f32, tag="ccs")
            nc.vector.scalar_tensor_tensor(
                cc_s[0:1, 0:1], alp[0:1, 2:3], -math.log(DT), lgm[0:1, 0:1],
                OP.mult, OP.subtract)
            a = nc.scalar.activation(cc_s[0:1, 1:2], cc_s[0:1, 0:1], AF.Exp)
            act_late.append(a)

            # late pack: [beta, sigma, gamma, mu, gamma+mu, -sigma,
            #             -(gamma+mu), C] -> broadcast [128, 8]
            sc16 = spool.tile([1, 16], f32, tag="sc16")
            nc.vector.tensor_copy(sc16[0:1, 0:4], sp[0:1, 0:4])
            nc.vector.tensor_tensor(sc16[0:1, 4:5], sp[0:1, 2:3],
                                    sp[0:1, 3:4], OP.add)      # gamma+mu
            nc.vector.tensor_scalar_mul(sc16[0:1, 5:6], sp[0:1, 1:2], -1.0)
            nc.vector.tensor_scalar_mul(sc16[0:1, 6:7], sc16[0:1, 4:5], -1.0)
            nc.vector.tensor_copy(sc16[0:1, 7:8], cc_s[0:1, 1:2])
            scb = cpool.tile([128, 8], f32)
            nc.gpsimd.partition_broadcast(scb[:], sc16[0:1, 0:8])
            beta128 = scb[:, 0:1]
            sig128 = scb[:, 1:2]
            gam128 = scb[:, 2:3]
            mu128 = scb[:, 3:4]
            nsig128 = scb[:, 5:6]
            ngpm128 = scb[:, 6:7]
            c128 = scb[:, 7:8]

            # ------- P1b: shifted Caputo kernel values wbuf -------
            # m(v) = v - 1152 + 128*d ; w1[m] = m^e - (m-1)^e, 1<=m<=8191
            wtmp = tc.tile_pool(name="wtmp", bufs=1)
            with wtmp as wt:
                vi = wt.tile([128, WBC], i32, tag="vi")
                nc.gpsimd.iota(vi[:], [[1, WBC]], channel_multiplier=WBC)
                mf = wt.tile([128, WBC], f32, tag="mf")
                nc.vector.tensor_copy(mf[:], vi[:])   # cast int -> f32
                nc.vector.tensor_scalar(mf[:], mf[:], shift128, None, OP.add)
                # masks
                mk1 = wt.tile([128, WBC], f32, tag="mk1")
                nc.vector.tensor_scalar(mk1[:], mf[:], 0.0, 1.0, OP.max,
                                        OP.min)
                mk2 = wt.tile([128, WBC], f32, tag="mk2")
                nc.vector.tensor_scalar(mk2[:], mf[:], -1.0, None, OP.add)
                nc.vector.tensor_scalar(mk2[:], mk2[:], 0.0, 1.0, OP.max,
                                        OP.min)
                mk3 = wt.tile([128, WBC], f32, tag="mk3")
                nc.vector.tensor_scalar(mk3[:], mf[:], -1.0, 8192.0, OP.mult,
                                        OP.add)
                nc.vector.tensor_scalar(mk3[:], mk3[:], 0.0, 1.0, OP.max,
                                        OP.min)
                # p1 = exp(e * ln(max(m,1)))
                p1 = wt.tile([128, WBC], f32, tag="p1")
                nc.vector.tensor_scalar(p1[:], mf[:], 1.0, None, OP.max)
                a = nc.scalar.activation(p1[:], p1[:], AF.Ln)
                act_late.append(a)
                act_p1a = nc.scalar.activation(p1[:], p1[:], AF.Exp,
                                               scale=e128)  # noqa: F841
                # p2 = exp(e * ln(max(m-1,1)))
                p2 = wt.tile([128, WBC], f32, tag="p2")
                nc.vector.tensor_scalar(p2[:], mf[:], -1.0, 1.0, OP.add,
                                        OP.max)
                a = nc.scalar.activation(p2[:], p2[:], AF.Ln)
                act_late.append(a)
                act_p1b = nc.scalar.activation(p2[:], p2[:], AF.Exp,
                                               scale=e128)
                # w1 = (p1*mk1 - p2*mk2) * mk3
                nc.vector.tensor_tensor(p1[:], p1[:], mk1[:], OP.mult)
                nc.vector.tensor_tensor(p2[:], p2[:], mk2[:], OP.mult)
                nc.vector.tensor_tensor(p1[:], p1[:], p2[:], OP.subtract)
                nc.vector.tensor_tensor(p1[:], p1[:], mk3[:], OP.mult)
                wbf = wt.tile([128, WBC], bf16, tag="wbf")
                nc.vector.tensor_copy(wbf[:], p1[:])
                nc.sync.dma_start(
                    wbuf_dram[:].rearrange("(p f) -> p f", p=128), wbf[:])

            # G2[s', u] = wbuf[u + s' + 1]  == Wmega[127-s', u]
            # (contraction dim pre-reversed; dpsi gets partition-reversed
            #  on-chip by a J128 exchange matmul to match)
            wmega = cpool.tile([128, WMC], bf16)
            src = bass.AP(
                tensor=wbuf_dram[:].tensor, offset=1,
                ap=[[1, 128], [1, WMC]])
            nc.sync.dma_start(wmega[:], src)

            # ------- P2: MLP (tanh/exp table set) -------
            hT = [apool.tile([128, KT * RLOC], f16, tag="hA", name="hA"),
                  apool.tile([128, KT * RLOC], f16, tag="hB", name="hB")]
            with tc.tile_pool(name="psum_mlp", bufs=1, space="PSUM") as pmm:
                # layer 0: outer product W_in^T (x) t
                for mt in range(KT):
                    for c0, cw in CHUNKS:
                        ps = pmm.tile([128, 512], f32, tag="mlp", name="ps", bufs=5)
                        nc.tensor.matmul(
                            ps[:, 0:cw],
                            win_sb[0:1, mt * 128:(mt + 1) * 128],
                            tsh_sb[0:1, c0:c0 + cw],
                            start=True, stop=True)
                        nc.scalar.activation(
                            hT[0][:, mt * RLOC + c0:mt * RLOC + c0 + cw],
                            ps[:, 0:cw], AF.Tanh, bias=binp_sb[:, mt:mt + 1])
                # hidden layers
                for l in range(DEPTH - 1):
                    src_t, dst_t = hT[l % 2], hT[(l + 1) % 2]
                    for c0, cw in CHUNKS:
                        for mt in range(KT):
                            ps = pmm.tile([128, 512], f32, tag="mlp", name="ps", bufs=5)
                            for kt in range(KT):
                                nc.tensor.matmul(
                                    ps[:, 0:cw],
                                    wh_sb[l][:, kt * H + mt * 128:
                                             kt * H + mt * 128 + 128],
                                    src_t[:, kt * RLOC + c0:
                                          kt * RLOC + c0 + cw],
                                    start=(kt == 0), stop=(kt == KT - 1))
                            nc.scalar.activation(
                                dst_t[:, mt * RLOC + c0:mt * RLOC + c0 + cw],
                                ps[:, 0:cw], AF.Tanh,
                                bias=bhp_sb[:, l * KT + mt:l * KT + mt + 1])

                # output layer -> ez = exp(z + b_out), unnormalized
                hlast = hT[(DEPTH - 1) % 2]
                ezT = apool.tile([5, RLOC], f32r, tag="ezT")
                for c0, cw in CHUNKS:
                    ps = pmm.tile([5, 512], f32, tag="zed", name="ps", bufs=1)
                    for kt in range(KT):
                        nc.tensor.matmul(
                            ps[:, 0:cw],
                            woutp_sb[:, kt * 5:(kt + 1) * 5],
                            hlast[:, kt * RLOC + c0:kt * RLOC + c0 + cw],
                            start=(kt == 0), stop=(kt == KT - 1))
                    act_ez = nc.scalar.activation(
                        ezT[:, c0:c0 + cw], ps[:, 0:cw], AF.Exp,
                        bias=bout5_sb[:, 0:1])

                # softmax denominators: column sums via ones-matmul
                ones5f = cpool.tile([5, 1], f32)
                nc.vector.memset(ones5f[:], 1.0)
                ones5 = cpool.tile([5, 1], f32r)
                nc.vector.tensor_copy(ones5[:], ones5f[:])
                ones1x5 = cpool.tile([1, 5], f32)
                nc.vector.memset(ones1x5[:], 1.0)
                rinv = apool.tile([1, RLOC], f32, tag="rinv")
                rscr = apool.tile([1, RLOC], f32, tag="rscr")
                ssum = apool.tile([1, RLOC], f32, tag="ssum")
                for c0, cw in CHUNKS:
                    ps = pmm.tile([1, 512], f32, tag="ssum", name="ps", bufs=1)
                    nc.tensor.matmul(
                        ps[:, 0:cw], ones5[:],
                        ezT[:, c0:c0 + cw],
                        start=True, stop=True)
                    nc.vector.tensor_copy(ssum[0:1, c0:c0 + cw], ps[:, 0:cw])
                nc.vector.reciprocal_approx_accurate(
                    rinv[0:1, :], ssum[0:1, :], rscr[0:1, :])

                # y^T = ez * rinv (replicate rinv to 5 partitions via matmul)
                yT = apool.tile([5, RLOC], f32, tag="yT")
                for c0, cw in CHUNKS:
                    ps = pmm.tile([5, 512], f32, tag="rrep", name="ps", bufs=1)
                    nc.tensor.matmul(
                        ps[:, 0:cw], ones1x5[:],
                        rinv[0:1, c0:c0 + cw],
                        start=True, stop=True)
                    nc.vector.tensor_tensor(
                        yT[:, c0:c0 + cw],
                        ezT[:, c0:c0 + cw], ps[:, 0:cw], OP.mult)

            from concourse.tile_rust import add_dep_helper
            for a in act_late:
                add_dep_helper(a.ins, act_ez.ins, sync=False,
                               reason="ln-set ACT after MLP ACT stream")

            # dpsi^T local; last col (global row 1024d+1023) fixed post-gather
            dpsiT = apool.tile([5, ROWS], f32, tag="dpsiT")
            nc.vector.tensor_tensor(dpsiT[:, 0:ROWS], yT[:, 1:ROWS + 1],
                                    yT[:, 0:ROWS], OP.subtract)

            # ------- P3: fold via PE transposes (all local) -------
            dloc = spool.tile([128, 40], f32, tag="dloc")
            yloc = spool.tile([128, 40], f32, tag="yloc")
            with tc.tile_pool(name="psum_fold", bufs=2, space="PSUM") as pf:
                for dst_sb, srcT in ((dloc, dpsiT), (yloc, yT)):
                    pt = pf.tile([128, 40], f32, tag="fold")
                    for j in range(NQ):
                        nc.tensor.transpose(
                            pt[:, j * 5:(j + 1) * 5],
                            srcT[:, j * 128:(j + 1) * 128],
                            ident5_sb[:],
                        )
                    nc.vector.tensor_copy(dst_sb[:], pt[:])

            # partition-reverse local dpsi (to match the pre-reversed
            # contraction dim of the Toeplitz band)
            dgb = spool.tile([128, 40], bf16, tag="dgb")
            nc.vector.tensor_copy(dgb[:], dloc[:])
            dgr = spool.tile([128, 40], bf16, tag="dgr")
            with tc.tile_pool(name="psum_rev", bufs=1, space="PSUM") as prv:
                pr = prv.tile([128, 40], f32, tag="rev")
                nc.tensor.matmul(pr[:], j128_sb[:], dgb[:],
                                 start=True, stop=True)
                nc.vector.tensor_copy(dgr[:], pr[:])


            # ------- P6: local partial Toeplitz conv over all 64 blocks ---
            with tc.tile_pool(name="psum_out", bufs=2, space="PSUM") as po:
                conv = po.tile([128, NB * 5], f32, tag="conv")
                # covering pass first (disjoint regions), then accumulate;
                # keeps each matmul region uniformly fresh-or-written
                ms = list(range(0, NB, NQ)) + [m for m in range(NB)
                                               if m % NQ != 0]
                for i, m in enumerate(ms):
                    nj = min(NQ, NB - m)
                    nc.tensor.matmul(
                        conv[:, 5 * m:5 * (m + nj)],
                        wmega[:, 128 * m:128 * (m + 1)],
                        dgr[:, 0:5 * nj],
                        start=(i == 0), stop=(i == len(ms) - 1))
                conv_sb = spool.tile([128, NB * 5], f32, tag="convsb")
                nc.vector.tensor_copy(conv_sb[:], conv[:])
                nc.sync.dma_start(
                    cc2_in[:].rearrange("(g p) f -> p g f", p=128),
                    conv_sb[:].rearrange("p (g f) -> p g f", g=NCORES))

            # ------- ReduceScatter: rank d receives its 8 blocks summed ----
            nc.gpsimd.collective_compute(
                "ReduceScatter", OP.add,
                replica_groups=[list(range(NCORES))],
                ins=[cc2_in[:].opt()], outs=[rs_out[:].opt()])
            rsb = spool.tile([128, 40], f32, tag="rsb")
            nc.sync.dma_start(rsb[:], rs_out[:])

            with tc.tile_pool(name="psum_loss", bufs=1, space="PSUM") as po:

                # f from local y; res = C*conv - f; partial = sum(res^2)
                yb4 = yloc[:].rearrange("p (q c) -> p q c", q=NQ)
                fb = spool.tile([128, 40], f32, tag="fb")
                fb4 = fb[:].rearrange("p (q c) -> p q c", q=NQ)
                t1 = spool.tile([128, NQ], f32, tag="t1")
                liv = spool.tile([128, NQ], f32, tag="liv")
                # living = 1 - y_d ; linv = 1/living
                nc.vector.tensor_scalar(liv[:], yb4[:, :, 4], -1.0, 1.0,
                                        OP.mult, OP.add)
                nc.vector.reciprocal(liv[:], liv[:])
                # inf = beta * s * i / living
                nc.vector.tensor_tensor(t1[:], yb4[:, :, 0], yb4[:, :, 2],
                                        OP.mult)
                nc.vector.tensor_tensor(t1[:], t1[:], liv[:], OP.mult)
                nc.vector.tensor_scalar(t1[:], t1[:], beta128, None, OP.mult)
                # f0 = -inf
                nc.vector.tensor_scalar(fb4[:, :, 0], t1[:], -1.0, None,
                                        OP.mult)
                # f1 = inf - sigma*e
                nc.vector.scalar_tensor_tensor(
                    fb4[:, :, 1], yb4[:, :, 1], nsig128, t1[:],
                    OP.mult, OP.add)
                # f2 = sigma*e - (gamma+mu)*i
                nc.vector.tensor_scalar(t1[:], yb4[:, :, 1], sig128, None,
                                        OP.mult)
                nc.vector.scalar_tensor_tensor(
                    fb4[:, :, 2], yb4[:, :, 2], ngpm128, t1[:],
                    OP.mult, OP.add)
                # f3 = gamma*i ; f4 = mu*i
                nc.vector.tensor_scalar(fb4[:, :, 3], yb4[:, :, 2], gam128,
                                        None, OP.mult)
                nc.vector.tensor_scalar(fb4[:, :, 4], yb4[:, :, 2], mu128,
                                        None, OP.mult)

                res = spool.tile([128, 40], f32, tag="res")
                nc.vector.scalar_tensor_tensor(res[:], rsb[:], c128, fb[:],
                                               OP.mult, OP.subtract)
                sq = spool.tile([128, 40], f32, tag="sq")
                rowsum = spool.tile([128, 1], f32, tag="rowsum")
                nc.vector.scalar_tensor_tensor(
                    sq[:], res[:], 0.0, res[:], OP.add, OP.mult,
                    accum_out=rowsum[:])

                ones128 = cpool.tile([128, 1], f32)
                nc.vector.memset(ones128[:], 1.0)
                ploss = po.tile([1, 1], f32, tag="ploss")
                nc.tensor.matmul(ploss[:], ones128[:], rowsum[:],
                                 start=True, stop=True)
                part_sb = spool.tile([1, 1], f32, tag="part")
                nc.scalar.mul(part_sb[:], ploss[:], 1.0 / (N * 5))

            # ------- P7: per-core partial loss out (host sums the 8) ----
            nc.sync.dma_start(out_d.ap(), part_sb[:])

    nc.compile()
    return nc


def _in_maps(inputs):
    t = np.asarray(inputs["t"], np.float32)
    W_in = np.asarray(inputs["W_in"], np.float32)
    b_in = np.asarray(inputs["b_in"], np.float32)
    Wh = np.asarray(inputs["Wh"], np.float32)
    bh = np.asarray(inputs["bh"], np.float32)
    W_out = np.asarray(inputs["W_out"], np.float32)
    b_out = np.asarray(inputs["b_out"], np.float32)

    whp = np.ascontiguousarray(
        Wh.reshape(DEPTH - 1, KT, 128, H).transpose(2, 0, 1, 3)
        .reshape(128, (DEPTH - 1) * KT * H))
    binp = np.ascontiguousarray(b_in.reshape(KT, 128).T)
    bhp = np.ascontiguousarray(
        bh.reshape(DEPTH - 1, KT, 128).transpose(2, 0, 1)
        .reshape(128, (DEPTH - 1) * KT))
    woutp = np.ascontiguousarray(
        W_out.reshape(KT, 128, 5).transpose(1, 0, 2).reshape(128, KT * 5))
    params = np.zeros((1, 8), np.float32)
    params[0, 0] = inputs["raw_beta"][0]
    params[0, 1] = inputs["raw_sigma"][0]
    params[0, 2] = inputs["raw_gamma"][0]
    params[0, 3] = inputs["raw_mu"][0]
    params[0, 4] = inputs["z_alpha"][0]

    maps = []
    for d in range(NCORES):
        maps.append({
            "tsh": np.ascontiguousarray(
                np.concatenate([
                    t[d * ROWS:min(N, d * ROWS + RLOC), 0],
                    np.repeat(t[N - 1:N, 0],
                              max(0, d * ROWS + RLOC - N))]).reshape(1, RLOC)),
            "win": np.ascontiguousarray(W_in.reshape(1, H)),
            "binp": binp,
            "whp": whp,
            "bhp": bhp,
            "woutp": woutp,
            "bout5": np.ascontiguousarray(b_out.reshape(5, 1)),
            "params": params,
            "coref": np.array([[-128.0 - 1024.0 * d, 0.0]], np.float32),
        })
    return maps


def kernel(**inputs) -> np.ndarray:
    from concourse.bass_utils import run_bass_kernel_spmd

    if "nc" not in _CACHE:
        _CACHE["nc"] = _build()
    nc = _CACHE["nc"]
    res = run_bass_kernel_spmd(nc, _in_maps(inputs), list(range(NCORES)))
    total = np.float32(0.0)
    for r in res.results:
        total = np.float32(total + np.asarray(r["out"], np.float32)[0, 0])
    return np.asarray(total, np.float32).reshape(())
